# revision 1
# baseline (speedup 1.0000x reference)
"""Trainium2 Bass kernel for single-head MHA (B=32, G=1024, D=256), data-parallel
over batch across 8 NeuronCores.

Per-core algorithm (BPC=4 batches/core), all layouts chosen so no G x G
transposes are ever needed:

  dT   = data_b^T                  [D, G]   (PE transposes of 128x128 tiles)
  QT   = Wq @ dT                   [D, G]   (lhsT=WqT chunk, rhs=dT)
  KT   = Wk @ dT                   [D, G]
  V    = data_b @ Wv^T             [G, D]   (lhsT=dT chunk, rhs=WvT)
  ST   = K @ Q^T  (= S^T)          [G, G]   per k-tile of 128 rows
  PT   = exp(NORM*ST + bias_k)     bias_k = -100 * mask[k]  (per-partition bias
                                   on ScalarE; exp(-100) == 0 exactly)
  HT   = V^T @ PT (via lhsT=V chunk, rhs=PT)   [D, G]
  l    = ones^T @ PT               [1, G]   row sums of PT = softmax denominators
  F    = H^T^T @ WoT               [G, D]   (lhsT=HT chunk, rhs=WoT)
  out  = F * (1/l)[q] + b_out      (one scalar_tensor_tensor on VectorE)

Masking correctness vs reference: reference fills masked logits with -30 and
re-zeroes attn post-softmax; its denominator keeps exp(-30 - max) ~ 1e-13
contributions which are below fp32 resolution of the sum. We use exp(-100) = 0.
"""

import math

import numpy as np

import concourse.bass as bass
import concourse.mybir as mybir
import concourse.tile as tile
import concourse.bass_isa as bass_isa
from concourse import bacc
from concourse.bass_utils import run_bass_kernel_spmd
from concourse.masks import make_identity

N_CORES = 8
B = 32
G = 1024
D = 256
BPC = B // N_CORES          # batches per core
TOK = BPC * G               # tokens per core
NORM = 1.0 / math.sqrt(D)
MASK_BIAS = -100.0

F32 = mybir.dt.float32
F32R = mybir.dt.float32r
I32 = mybir.dt.int32
BF16 = mybir.dt.bfloat16

KD = G // 128               # 8 k-tiles (and q-tiles) per batch
DT_CH = D // 128            # 2 chunks of the feature dim


def build_program(mm_mode: str = "f32", bpc: int = BPC, enable_asserts: bool = False,
                  reps: int = 1):
    """Build + schedule + compile the per-core SPMD program.

    mm_mode: "f32" (exact, 4 cyc/row), "f32r" (fp32 data, fast PE mode,
             1 cyc/row at N>=256), "bf16" (operands cast to bf16).
    reps: if > 1, wrap the whole body in a hardware loop re-executing it —
          used only for benchmarking (slope timing past the dispatch
          overhead of the axon tunnel).
    """
    assert mm_mode in ("f32", "f32r", "bf16")
    # storage dtype of all matmul operand tiles; fp32r operands must be
    # produced pre-rounded (walrus birverifier enforces this), so the tiles
    # are declared float32r and every PSUM->SBUF copy/activation rounds.
    st_dt = {"bf16": BF16, "f32r": F32R, "f32": F32}[mm_mode]

    def mm(ap):
        return ap

    nc = bacc.Bacc(
        "TRN2",
        target_bir_lowering=False,
        debug=False,
        enable_asserts=enable_asserts,
    )

    tok = bpc * G
    data_d = nc.dram_tensor("data", [tok, D], F32, kind="ExternalInput").ap()
    mask_d = nc.dram_tensor("mask", [bpc, G], I32, kind="ExternalInput").ap()
    wq_d = nc.dram_tensor("w_query", [D, D], F32, kind="ExternalInput").ap()
    wk_d = nc.dram_tensor("w_key", [D, D], F32, kind="ExternalInput").ap()
    wv_d = nc.dram_tensor("w_val", [D, D], F32, kind="ExternalInput").ap()
    wo_d = nc.dram_tensor("w_out", [D, D], F32, kind="ExternalInput").ap()
    b_d = nc.dram_tensor("b_out", [D], F32, kind="ExternalInput").ap()
    out_d = nc.dram_tensor("out", [tok, D], F32, kind="ExternalOutput").ap()

    from contextlib import ExitStack
    with tile.TileContext(nc) as tc, ExitStack() as ctx:
        _attention_body(ctx, tc, out_d, data_d, mask_d, wq_d, wk_d, wv_d,
                        wo_d, b_d, mm, st_dt, bpc, reps)

    nc.compile()
    return nc


def OUT_ENG(nc):
    return nc.sync


def _attention_body(ctx, tc, out_d, data_d, mask_d, wq_d, wk_d, wv_d, wo_d, b_d,
                    mm, st_dt, bpc, reps=1):
    nc = tc.nc

    const = ctx.enter_context(tc.tile_pool(name="const", bufs=1))
    wpool = ctx.enter_context(tc.tile_pool(name="wpool", bufs=1))
    def _bufs(name, dflt):
        return dflt
    dnat_p = ctx.enter_context(tc.tile_pool(name="dnat", bufs=_bufs("DNAT", 10)))
    dT_p = ctx.enter_context(tc.tile_pool(name="dT", bufs=_bufs("DT", 3)))
    qt_p = ctx.enter_context(tc.tile_pool(name="qt", bufs=_bufs("QT", 3)))
    kt_p = ctx.enter_context(tc.tile_pool(name="kt", bufs=_bufs("KT", 3)))
    v_p = ctx.enter_context(tc.tile_pool(name="v", bufs=_bufs("V", 9)))
    pt_p = ctx.enter_context(tc.tile_pool(name="pt", bufs=_bufs("PT", 9)))
    ht_p = ctx.enter_context(tc.tile_pool(name="ht", bufs=_bufs("HT", 2)))
    out_p = ctx.enter_context(tc.tile_pool(name="outp", bufs=_bufs("OUT", 8)))
    misc_p = ctx.enter_context(tc.tile_pool(name="misc", bufs=_bufs("MISC", 3)))

    ACT_COPIES = True
    QT_ON_ACT = True
    W_ON_SCALAR = False
    LROW_ON_ACT = True
    sm_bufs = 5
    l_bufs = 1
    ps_sm = ctx.enter_context(tc.tile_pool(name="ps_sm", bufs=sm_bufs, space="PSUM"))
    acc_bufs = 2
    ps_acc = ctx.enter_context(tc.tile_pool(name="ps_acc", bufs=acc_bufs,
                                            space="PSUM"))
    ps_l = ctx.enter_context(tc.tile_pool(name="ps_l", bufs=l_bufs, space="PSUM"))

    # ---- constants ----------------------------------------------------------
    ident = const.tile([128, 128], F32, tag="ident")
    make_identity(nc, ident)

    ones_f32 = const.tile([128, 1], F32, tag="ones_f32")
    nc.vector.memset(ones_f32, 1.0)
    ones = const.tile([128, 1], st_dt, tag="ones")
    nc.vector.tensor_copy(ones, ones_f32)

    # Dummy exp as ScalarE's first instruction: pulls the ~2.7us
    # ACT_TABLE_LOAD of the exp_and_others set (which also covers Copy) into
    # the DMA prologue instead of stalling the first attention tile, and
    # prevents a second mid-kernel table switch.
    act_warm = const.tile([128, 1], F32, tag="act_warm")
    nc.scalar.activation(out=act_warm, in_=ones_f32,
                         func=mybir.ActivationFunctionType.Exp)

    bias_rep = const.tile([128, D], F32, tag="bias_rep")
    b_bcast = bass.AP(tensor=b_d.tensor, offset=b_d.offset,
                      ap=[[0, 128]] + list(b_d.ap))
    nc.gpsimd.dma_start(out=bias_rep, in_=b_bcast)

    # ---- weight transposes: W [d_out, d_in] -> WT chunks [128 (d_in), D] ----
    wT = {}
    wnat_all = {}
    for name, w_d in (("q", wq_d), ("k", wk_d), ("v", wv_d), ("o", wo_d)):
        wnat = []
        for r in range(DT_CH):
            t = wpool.tile([128, D], F32, tag=f"wnat_{name}{r}",
                           name=f"wnat_{name}{r}")
            nc.sync.dma_start(out=t, in_=w_d[r * 128:(r + 1) * 128, :])
            wnat.append(t)
        wnat_all[name] = wnat
        if name == "o":
            chunks = []
            for c in range(DT_CH):
                wt_c = wpool.tile([128, D], F32, tag=f"wT_{name}{c}",
                                  name=f"wT_{name}{c}")
                for r in range(DT_CH):
                    ps = ps_sm.tile([128, 512], F32, tag="ps_sm",
                                    name=f"psw{name}{c}{r}")
                    nc.tensor.transpose(
                        ps[:, :128], wnat[r][:, c * 128:(c + 1) * 128], ident)
                    nc.scalar.copy(wt_c[:, r * 128:(r + 1) * 128], ps[:, :128])
                chunks.append(wt_c)
            wT[name] = chunks

    # NT = Wq^T @ Wk  [j, i]: folds both attention projections into one.
    # S^T = data (Wk^T Wq) data^T, so BT = NT @ dT replaces QT, and the
    # stationary side of S^T becomes dT itself (KT is never built).
    nt_chunks = []
    for jt in range(DT_CH):
        ps = ps_sm.tile([128, 512], F32, tag="ps_sm", name=f"psnt{jt}")
        for dc in range(DT_CH):
            # plain-f32 matmul (DMA-fed operands may not feed fp32r mode);
            # one-time cost, the output copy rounds to the storage dtype
            nc.tensor.matmul(
                ps[:, :D],
                wnat_all["q"][dc][:, jt * 128:(jt + 1) * 128],
                wnat_all["k"][dc],
                start=(dc == 0), stop=(dc == DT_CH - 1))
        ntc = wpool.tile([128, D], st_dt, tag=f"nt{jt}", name=f"nt{jt}")
        nc.scalar.copy(ntc, ps[:, :D])
        nt_chunks.append(ntc)
    wT["q"] = nt_chunks

    # P^T = Wv^T @ Wo^T: folds the value and output projections, so the
    # per-batch V "projection" becomes a plain rounding copy of data tiles
    # (out = attn @ data @ P^T + b)
    pto_chunks = []
    for dtile in range(DT_CH):
        ps = ps_sm.tile([128, 512], F32, tag="ps_sm", name=f"pspt{dtile}")
        for mc in range(DT_CH):
            nc.tensor.matmul(
                ps[:, :D],
                wnat_all["v"][mc][:, dtile * 128:(dtile + 1) * 128],
                wT["o"][mc],
                start=(mc == 0), stop=(mc == DT_CH - 1))
        ptoc = wpool.tile([128, D], st_dt, tag=f"pto{dtile}", name=f"pto{dtile}")
        nc.scalar.copy(ptoc, ps[:, :D])
        pto_chunks.append(ptoc)
    wT["o"] = pto_chunks

    # ---- staged per-batch pipeline -----------------------------------------
    # stage A: mask prep + data load + transpose + Q/K/V projections
    # stage B: per k-tile S^T -> exp -> (pipelined) PV + l accumulation
    # stage C: 1/l + final projection + epilogue + store
    # Emission order interleaves A two batches ahead so the in-order PE queue
    # always has dense work while stage C waits on the l -> 1/l chain.

    state = {}

    def stage_a(b):
        row0 = b * G
        mb8 = misc_p.tile([KD, 128], I32, tag="mb8", name=f"mb8_{b}")
        nc.sync.dma_start(out=mb8, in_=mask_d[b].rearrange("(j f) -> j f", j=KD))
        mbf = misc_p.tile([KD, 128], F32, tag="mbf", name=f"mbf_{b}")
        nc.vector.tensor_scalar_mul(mbf, mb8, MASK_BIAS)
        ps_mb = ps_sm.tile([128, 512], F32, tag="ps_sm", name=f"psmb_{b}")
        nc.tensor.transpose(ps_mb[:, :KD], mbf, ident[:KD, :KD])
        mbT = misc_p.tile([128, KD], F32, tag="mbT", name=f"mbT_{b}")
        nc.vector.tensor_copy(mbT, ps_mb[:, :KD])

        dnat = []
        for t in range(KD):
            dn = dnat_p.tile([128, D], F32, tag="dnat", name=f"dn_{b}_{t}")
            # alternate HWDGE (sync) / SWDGE (gpsimd) so the two DMA paths
            # stream data tiles in parallel
            (nc.sync if t % 2 == 0 else nc.gpsimd).dma_start(
                out=dn, in_=data_d[row0 + t * 128:row0 + (t + 1) * 128, :])
            dnat.append(dn)
        dT = []
        for c in range(DT_CH):
            dc = dT_p.tile([128, G], st_dt, tag=f"dT{c}", name=f"dT_{b}_{c}")
            for g in range(KD // 4):
                ps = ps_sm.tile([128, 512], F32, tag="ps_sm", name=f"psdt_{b}_{c}_{g}")
                for j in range(4):
                    t = g * 4 + j
                    nc.tensor.transpose(ps[:, j * 128:(j + 1) * 128],
                                        dnat[t][:, c * 128:(c + 1) * 128], ident)
                if ACT_COPIES and g % 2 == 0:
                    nc.scalar.copy(dc[:, g * 512:(g + 1) * 512], ps)
                else:
                    nc.vector.tensor_copy(dc[:, g * 512:(g + 1) * 512], ps)
            dT.append(dc)

        QT, KT = [], []
        for wname, dest, pool in (("q", QT, qt_p),):
            for dt_i in range(DT_CH):
                dst = pool.tile([128, G], st_dt, tag=f"{wname}T{dt_i}",
                                name=f"{wname}T_{b}_{dt_i}")
                for h in range(2):
                    ps = ps_sm.tile([128, 512], F32, tag="ps_sm",
                                    name=f"ps{wname}_{b}_{dt_i}_{h}")
                    for ic in range(DT_CH):
                        nc.tensor.matmul(
                            ps,
                            mm(wT[wname][ic][:, dt_i * 128:(dt_i + 1) * 128]),
                            mm(dT[ic][:, h * 512:(h + 1) * 512]),
                            start=(ic == 0), stop=(ic == DT_CH - 1))
                    if ACT_COPIES and QT_ON_ACT and wname == "q":
                        nc.scalar.copy(dst[:, h * 512:(h + 1) * 512], ps)
                    else:
                        nc.vector.tensor_copy(dst[:, h * 512:(h + 1) * 512], ps)
                dest.append(dst)

        V = []
        for kt_i in range(KD):
            vt = v_p.tile([128, D], st_dt, tag="v", bufs=18,
                          name=f"v_{b}_{kt_i}")
            nc.vector.tensor_copy(vt, dnat[kt_i])
            V.append(vt)
        state[b] = {"QT": QT, "KT": dT, "V": V, "mbT": mbT}

    def stage_b(b):
        st = state[b]
        QT, KT, V, mbT = st["QT"], st["KT"], st["V"], st["mbT"]
        PT = [None] * KD
        HT = [ht_p.tile([128, G], st_dt, tag=f"hT{i}", name=f"hT_{b}_{i}")
              for i in range(DT_CH)]
        l_row = misc_p.tile([1, G], F32, tag="l_row", name=f"lrow_{b}")

        def emit_s(kt_i):
            pt = pt_p.tile([128, G], st_dt, tag="pt", name=f"pt_{b}_{kt_i}")
            for h in range(2):
                ps = ps_sm.tile([128, 512], F32, tag="ps_sm",
                                name=f"pss_{b}_{kt_i}_{h}")
                for dt_i in range(DT_CH):
                    nc.tensor.matmul(
                        ps,
                        mm(KT[dt_i][:, kt_i * 128:(kt_i + 1) * 128]),
                        mm(QT[dt_i][:, h * 512:(h + 1) * 512]),
                        start=(dt_i == 0), stop=(dt_i == DT_CH - 1))
                nc.scalar.activation(
                    out=pt[:, h * 512:(h + 1) * 512], in_=ps,
                    func=mybir.ActivationFunctionType.Exp,
                    bias=mbT[:, kt_i:kt_i + 1], scale=NORM)
            PT[kt_i] = pt

        def pv_pass(h):
            psH = [ps_acc.tile([128, 512], F32, tag="ps_acc",
                               name=f"psH_{b}_{h}_{i}") for i in range(DT_CH)]
            def emit_pv(kt_i):
                for dt_i in range(DT_CH):
                    nc.tensor.matmul(
                        psH[dt_i],
                        mm(V[kt_i][:, dt_i * 128:(dt_i + 1) * 128]),
                        mm(PT[kt_i][:, h * 512:(h + 1) * 512]),
                        start=(kt_i == 0), stop=(kt_i == KD - 1))
            return psH, emit_pv

        # ---- pass h=0: S/exp production pipelined with PV h0 ----
        psH0, emit_pv0 = pv_pass(0)
        emit_s(0)
        for kt_i in range(1, KD):
            emit_s(kt_i)
            emit_pv0(kt_i - 1)
        emit_pv0(KD - 1)

        # l half 0 (PE) runs while DVE copies HT h0 out of the accumulators
        psl0 = ps_l.tile([1, 512], F32, tag="ps_l", name=f"psl_{b}_0")
        for kt_i in range(KD):
            nc.tensor.matmul(psl0, mm(ones), mm(PT[kt_i][:, 0:512]),
                             start=(kt_i == 0), stop=(kt_i == KD - 1))
        for dt_i in range(DT_CH):
            nc.vector.tensor_copy(HT[dt_i][:, 0:512], psH0[dt_i])

        # ---- pass h=1 ----
        psH1, emit_pv1 = pv_pass(1)
        for kt_i in range(KD):
            emit_pv1(kt_i)
        (nc.scalar.copy if LROW_ON_ACT else nc.vector.tensor_copy)(l_row[:, 0:512], psl0)
        psl1 = ps_l.tile([1, 512], F32, tag="ps_l", name=f"psl_{b}_1")
        for kt_i in range(KD):
            nc.tensor.matmul(psl1, mm(ones), mm(PT[kt_i][:, 512:1024]),
                             start=(kt_i == 0), stop=(kt_i == KD - 1))
        for dt_i in range(DT_CH):
            nc.vector.tensor_copy(HT[dt_i][:, 512:1024], psH1[dt_i])
        (nc.scalar.copy if LROW_ON_ACT else nc.vector.tensor_copy)(l_row[:, 512:1024], psl1)

        ps_inv = ps_sm.tile([128, 512], F32, tag="ps_sm", name=f"psinv_{b}")
        for j in range(KD):
            nc.tensor.transpose(
                ps_inv[:, j:j + 1], l_row[:, j * 128:(j + 1) * 128], ident[:1, :1])
        invl = misc_p.tile([128, KD], F32, tag="invl", name=f"invl_{b}")
        nc.vector.reciprocal(invl, ps_inv[:, :KD])
        st["HT"] = HT
        st["invl"] = invl

    def stage_c(b):
        st = state[b]
        HT, invl = st["HT"], st["invl"]
        row0 = b * G

        def emit_c_pair(p_i):
            ps = ps_sm.tile([128, 512], F32, tag="ps_sm", name=f"psf_{b}_{p_i}")
            for j in range(2):
                qt_i = p_i * 2 + j
                for dt_i in range(DT_CH):
                    nc.tensor.matmul(
                        ps[:, j * D:(j + 1) * D],
                        mm(HT[dt_i][:, qt_i * 128:(qt_i + 1) * 128]),
                        mm(wT["o"][dt_i]),
                        start=(dt_i == 0), stop=(dt_i == DT_CH - 1))
            for j in range(2):
                qt_i = p_i * 2 + j
                ot = out_p.tile([128, D], F32, tag="outp", name=f"ot_{b}_{qt_i}")
                nc.vector.scalar_tensor_tensor(
                    out=ot, in0=ps[:, j * D:(j + 1) * D],
                    scalar=invl[:, qt_i:qt_i + 1], in1=bias_rep,
                    op0=mybir.AluOpType.mult, op1=mybir.AluOpType.add)
                OUT_ENG(nc).dma_start(
                    out=out_d[row0 + qt_i * 128:row0 + (qt_i + 1) * 128, :], in_=ot)

        for p_i in range(KD // 2):
            emit_c_pair(p_i)
        del state[b]

    if reps > 1:
        loop_cm = tc.For_i(0, reps, 1)
        loop_cm.__enter__()

    # pipelined emission: stage A of the next batch is emitted between B(b)
    # and C(b) so the in-order PE queue has dense work while C waits on the
    # l -> 1/l chain
    stage_a(0)
    for b in range(bpc):
        stage_b(b)
        if b + 1 < bpc:
            stage_a(b + 1)
        stage_c(b)

    if reps > 1:
        loop_cm.__exit__(None, None, None)


# ---------------------------------------------------------------------------
# Runner: a cached jax.jit(shard_map) over the 8 cores, mirroring
# concourse.bass2jax.run_bass_via_pjrt but built once and reused so repeat
# calls pay only input transfer + execute (no retrace / recompile).
_RUNNER_CACHE = {}


def _make_runner(mm_mode):
    import jax
    from jax.experimental.shard_map import shard_map
    from jax.sharding import Mesh, NamedSharding, PartitionSpec

    from concourse.bass2jax import (
        _bass_exec_p,
        install_neuronx_cc_hook,
        partition_id_tensor,
    )

    nc = build_program(mm_mode)
    install_neuronx_cc_hook()
    assert nc.dbg_addr is None
    partition_name = (nc.partition_id_tensor.name
                      if nc.partition_id_tensor else None)

    in_names, out_names, out_avals, zero_outs = [], [], [], []
    for alloc in nc.m.functions[0].allocations:
        if not isinstance(alloc, mybir.MemoryLocationSet):
            continue
        name = alloc.memorylocations[0].name
        if alloc.kind == "ExternalInput":
            if name != partition_name:
                in_names.append(name)
        elif alloc.kind == "ExternalOutput":
            shape = tuple(alloc.tensor_shape)
            dtype = mybir.dt.np(alloc.dtype)
            out_names.append(name)
            out_avals.append(jax.core.ShapedArray(shape, dtype))
            zero_outs.append(np.zeros((N_CORES * shape[0],) + shape[1:], dtype))
    n_params = len(in_names)
    all_in_names = list(in_names) + list(out_names)
    if partition_name is not None:
        all_in_names.append(partition_name)

    def _body(*args):
        operands = list(args)
        if partition_name is not None:
            operands.append(partition_id_tensor())
        outs = _bass_exec_p.bind(
            *operands,
            out_avals=tuple(out_avals),
            in_names=tuple(all_in_names),
            out_names=tuple(out_names),
            lowering_input_output_aliases=(),
            sim_require_finite=False,
            sim_require_nnan=False,
            nc=nc,
        )
        return tuple(outs)

    devices = jax.devices()[:N_CORES]
    mesh = Mesh(np.asarray(devices), ("core",))
    in_specs = (PartitionSpec("core"),) * (n_params + len(out_names))
    out_specs = (PartitionSpec("core"),) * len(out_names)
    sharded = jax.jit(
        shard_map(_body, mesh=mesh, in_specs=in_specs, out_specs=out_specs,
                  check_rep=False),
        keep_unused=True,
    )
    sharding = NamedSharding(mesh, PartitionSpec("core"))
    dev_zeros = [jax.device_put(z, sharding) for z in zero_outs]
    return {
        "nc": nc, "fn": sharded, "in_names": in_names,
        "out_names": out_names, "sharding": sharding, "dev_zeros": dev_zeros,
    }


def get_runner(mm_mode=None):
    key = mm_mode or MM_MODE
    if key not in _RUNNER_CACHE:
        _RUNNER_CACHE[key] = _make_runner(key)
    return _RUNNER_CACHE[key]


MM_MODE = "f32r"


def _concat_inputs(data, mask, wq, wk, wv, wo, b):
    """Per-core shards concatenated on axis 0, keyed by dram tensor name."""
    return {
        "data": data,                                   # already [8*TOK, D]
        "mask": mask,                                   # [8*BPC, G]
        "w_query": np.concatenate([wq] * N_CORES, axis=0),
        "w_key": np.concatenate([wk] * N_CORES, axis=0),
        "w_val": np.concatenate([wv] * N_CORES, axis=0),
        "w_out": np.concatenate([wo] * N_CORES, axis=0),
        "b_out": np.concatenate([b] * N_CORES, axis=0),
    }


def kernel(data, mask, graph_size, evaluate, W_query, W_key, W_val, W_out, b_out,
           **_ignored):
    data = np.ascontiguousarray(np.asarray(data, dtype=np.float32))
    mask = np.ascontiguousarray(np.asarray(mask, dtype=np.int32))
    wq = np.ascontiguousarray(np.asarray(W_query, dtype=np.float32))
    wk = np.ascontiguousarray(np.asarray(W_key, dtype=np.float32))
    wv = np.ascontiguousarray(np.asarray(W_val, dtype=np.float32))
    wo = np.ascontiguousarray(np.asarray(W_out, dtype=np.float32))
    b = np.ascontiguousarray(np.asarray(b_out, dtype=np.float32))

    r = get_runner()
    cat = _concat_inputs(data, mask, wq, wk, wv, wo, b)
    args = [cat[n] for n in r["in_names"]] + list(r["dev_zeros"])
    outs = r["fn"](*args)
    out = np.asarray(outs[r["out_names"].index("out")])
    return out



# revision 2
# speedup vs baseline: 1.9366x; 1.9366x over previous
"""Trainium2 Bass kernel for single-head MHA (B=32, G=1024, D=256),
data-parallel over batch across 8 NeuronCores.

v2: bf16 matmul operands (1 cyc/row PE rate, half-size PSUM drains),
masked-key compaction (the host stable-sorts each batch's rows so unmasked
keys come first; only NKT=5 of 8 key tiles are computed — the per-partition
exp bias of -100 zeroes the masked tail inside tile NKT-1, and tiles
NKT..7 are entirely masked so they are skipped), one wide exp per key tile
([128,1024] across a 2-bank PSUM tile), and engine-balanced PSUM drains
(QT/HT/l on ScalarE, rest on VectorE).

Per-core per-batch algorithm (no GxG transposes anywhere):
  dT   = data_b^T              [D, G]  bf16 (PE transposes of dn16)
  QT   = NT @ dT               [D, G]  bf16 (NT = bf16(Wq^T Wk), folded QK)
  ST   = dT_k^T @ QT  (= S^T)  [128, G] f32 per key tile kt < NKT
  PT   = exp(NORM*ST + bias_k) bias_k = -100*mask[k] per-partition on ACT
  HT  += V_kt^T @ PT           [D, G]  V = dn16 (value proj folded into PTO)
  l    = ones^T @ PT           [1, G]
  F    = HT^T @ PTO            [G, D]  PTO = bf16(Wv^T Wo^T)
  out  = F * (1/l)[q] + b_out  (scalar_tensor_tensor on DVE)

Masking correctness vs reference: reference fills masked logits with -30
and re-zeroes attn post-softmax; its denominator keeps exp(-30 - max)
~1e-13 contributions which are below fp32 resolution of the sum. We use
exp(-100) = 0 and drop fully-masked key tiles entirely.
"""

import math

import numpy as np

import concourse.bass as bass
import concourse.mybir as mybir
import concourse.tile as tile
from concourse import bacc

N_CORES = 8
B = 32
G = 1024
D = 256
BPC = B // N_CORES
TOK = BPC * G
NORM = 1.0 / math.sqrt(D)
MASK_BIAS = -100.0
KD = G // 128                # 8 query tiles per batch
NKT = 5                      # key tiles computed (after compaction)
DT_CH = D // 128             # 2 chunks of the feature dim

F32 = mybir.dt.float32
I32 = mybir.dt.int32
BF16 = mybir.dt.bfloat16


def build_program(nkt: int = NKT, bpc: int = BPC, reps: int = 1,
                  enable_asserts: bool = False):
    assert 1 <= nkt <= KD
    nc = bacc.Bacc("TRN2", target_bir_lowering=False, debug=False,
                   enable_asserts=enable_asserts)

    tok = bpc * G
    data_d = nc.dram_tensor("data", [tok, D], F32, kind="ExternalInput").ap()
    mask_d = nc.dram_tensor("mask", [bpc, G], I32, kind="ExternalInput").ap()
    wq_d = nc.dram_tensor("w_query", [D, D], F32, kind="ExternalInput").ap()
    wk_d = nc.dram_tensor("w_key", [D, D], F32, kind="ExternalInput").ap()
    wv_d = nc.dram_tensor("w_val", [D, D], F32, kind="ExternalInput").ap()
    wo_d = nc.dram_tensor("w_out", [D, D], F32, kind="ExternalInput").ap()
    b_d = nc.dram_tensor("b_out", [D], F32, kind="ExternalInput").ap()
    out_d = nc.dram_tensor("out", [tok, D], F32, kind="ExternalOutput").ap()

    from contextlib import ExitStack
    with tile.TileContext(nc) as tc, ExitStack() as ctx:
        _body(ctx, tc, out_d, data_d, mask_d, wq_d, wk_d, wv_d, wo_d, b_d,
              nkt, bpc, reps)

    nc.compile()
    return nc


def _body(ctx, tc, out_d, data_d, mask_d, wq_d, wk_d, wv_d, wo_d, b_d,
          nkt, bpc, reps):
    nc = tc.nc
    from concourse.masks import make_identity

    const = ctx.enter_context(tc.tile_pool(name="const", bufs=1))
    wpool = ctx.enter_context(tc.tile_pool(name="wpool", bufs=1))
    dnat_p = ctx.enter_context(tc.tile_pool(name="dnat", bufs=10))
    dn16_p = ctx.enter_context(tc.tile_pool(name="dn16", bufs=18))
    dT_p = ctx.enter_context(tc.tile_pool(name="dT", bufs=3))
    qt_p = ctx.enter_context(tc.tile_pool(name="qt", bufs=3))
    pt_p = ctx.enter_context(tc.tile_pool(name="pt", bufs=2))
    ht_p = ctx.enter_context(tc.tile_pool(name="ht", bufs=2))
    out_p = ctx.enter_context(tc.tile_pool(name="outp", bufs=8))
    misc_p = ctx.enter_context(tc.tile_pool(name="misc", bufs=3))

    # PSUM banks: ps_t 1 + ps_q 2 + ps_s 2 + ps_acc 2 + ps_l 1 = 8
    ps_t = ctx.enter_context(tc.tile_pool(name="ps_t", bufs=1, space="PSUM"))
    ps_q = ctx.enter_context(tc.tile_pool(name="ps_q", bufs=1, space="PSUM"))
    ps_s = ctx.enter_context(tc.tile_pool(name="ps_s", bufs=1, space="PSUM"))
    ps_acc = ctx.enter_context(tc.tile_pool(name="ps_acc", bufs=2, space="PSUM"))
    ps_l = ctx.enter_context(tc.tile_pool(name="ps_l", bufs=1, space="PSUM"))

    # ---- constants ----------------------------------------------------------
    ident_f = const.tile([128, 128], F32, tag="identf")
    make_identity(nc, ident_f)
    ident = const.tile([128, 128], BF16, tag="ident")
    nc.vector.tensor_copy(ident, ident_f)

    ones_f32 = const.tile([128, 1], F32, tag="ones_f32")
    nc.vector.memset(ones_f32, 1.0)
    ones = const.tile([128, 1], BF16, tag="ones")
    nc.vector.tensor_copy(ones, ones_f32)

    # warm the exp table set before the first attention tile
    act_warm = const.tile([128, 1], F32, tag="act_warm")
    nc.scalar.activation(out=act_warm, in_=ones_f32,
                         func=mybir.ActivationFunctionType.Exp)

    bias_rep = const.tile([128, D], F32, tag="bias_rep")
    b_bcast = bass.AP(tensor=b_d.tensor, offset=b_d.offset,
                      ap=[[0, 128]] + list(b_d.ap))
    nc.gpsimd.dma_start(out=bias_rep, in_=b_bcast)

    # ---- weight prep: NT = bf16(Wq^T Wk), PTO = bf16(Wv^T Wo^T) -------------
    wnat = {}
    for name, w_d in (("q", wq_d), ("k", wk_d), ("v", wv_d), ("o", wo_d)):
        ts = []
        for r in range(DT_CH):
            t = wpool.tile([128, D], F32, tag=f"wnat_{name}{r}",
                           name=f"wnat_{name}{r}")
            nc.sync.dma_start(out=t, in_=w_d[r * 128:(r + 1) * 128, :])
            ts.append(t)
        wnat[name] = ts

    woT = []
    for c in range(DT_CH):
        wt_c = wpool.tile([128, D], F32, tag=f"woT{c}", name=f"woT{c}")
        for r in range(DT_CH):
            ps = ps_q.tile([128, 1024], F32, tag="ps_q", name=f"psw{c}{r}")
            nc.tensor.transpose(ps[:, :128],
                                wnat["o"][r][:, c * 128:(c + 1) * 128], ident_f)
            nc.scalar.copy(wt_c[:, r * 128:(r + 1) * 128], ps[:, :128])
        woT.append(wt_c)

    nt = []
    for jt in range(DT_CH):
        ps = ps_q.tile([128, 1024], F32, tag="ps_q", name=f"psnt{jt}")
        for dc in range(DT_CH):
            nc.tensor.matmul(
                ps[:, :D],
                wnat["q"][dc][:, jt * 128:(jt + 1) * 128],
                wnat["k"][dc],
                start=(dc == 0), stop=(dc == DT_CH - 1))
        t = wpool.tile([128, D], BF16, tag=f"nt{jt}", name=f"nt{jt}")
        nc.scalar.copy(t, ps[:, :D])
        nt.append(t)

    pto = []
    for dtile in range(DT_CH):
        ps = ps_q.tile([128, 1024], F32, tag="ps_q", name=f"pspt{dtile}")
        for mc in range(DT_CH):
            nc.tensor.matmul(
                ps[:, :D],
                wnat["v"][mc][:, dtile * 128:(dtile + 1) * 128],
                woT[mc],
                start=(mc == 0), stop=(mc == DT_CH - 1))
        t = wpool.tile([128, D], BF16, tag=f"pto{dtile}", name=f"pto{dtile}")
        nc.scalar.copy(t, ps[:, :D])
        pto.append(t)

    state = {}

    def stage_a(b):
        row0 = b * G
        mb8 = misc_p.tile([nkt, 128], I32, tag="mb8", name=f"mb8_{b}")
        nc.sync.dma_start(
            out=mb8,
            in_=mask_d[b][:nkt * 128].rearrange("(j f) -> j f", j=nkt))
        mbf = misc_p.tile([nkt, 128], F32, tag="mbf", name=f"mbf_{b}")
        nc.vector.tensor_scalar_mul(mbf, mb8, MASK_BIAS)
        ps_mb = ps_q.tile([128, 1024], F32, tag="ps_q", name=f"psmb_{b}")
        nc.tensor.transpose(ps_mb[:, :nkt], mbf, ident_f[:nkt, :nkt])
        mbT = misc_p.tile([128, nkt], F32, tag="mbT", name=f"mbT_{b}")
        nc.vector.tensor_copy(mbT, ps_mb[:, :nkt])

        dnat, dn16 = [], []
        for t in range(KD):
            dn = dnat_p.tile([128, D], F32, tag="dnat", name=f"dn_{b}_{t}")
            (nc.sync if t % 2 == 0 else nc.gpsimd).dma_start(
                out=dn, in_=data_d[row0 + t * 128:row0 + (t + 1) * 128, :])
            dnat.append(dn)
            d16 = dn16_p.tile([128, D], BF16, tag="dn16", name=f"dn16_{b}_{t}")
            nc.vector.tensor_copy(d16, dn)
            dn16.append(d16)

        dT = []
        for c in range(DT_CH):
            dc = dT_p.tile([128, G], BF16, tag=f"dT{c}", name=f"dT_{b}_{c}")
            pst = ps_t.tile([128, 1024], BF16, tag="ps_t", name=f"psdt_{b}_{c}")
            for g in range(KD):
                nc.tensor.transpose(pst[:, g * 128:(g + 1) * 128],
                                    dn16[g][:, c * 128:(c + 1) * 128], ident)
            nc.vector.tensor_copy(dc, pst)
            dT.append(dc)

        QT = []
        for dt_i in range(DT_CH):
            psq = ps_q.tile([128, 1024], F32, tag="ps_q", name=f"psq_{b}_{dt_i}")
            for h in range(2):
                for ic in range(DT_CH):
                    nc.tensor.matmul(
                        psq[:, h * 512:(h + 1) * 512],
                        nt[ic][:, dt_i * 128:(dt_i + 1) * 128],
                        dT[ic][:, h * 512:(h + 1) * 512],
                        start=(ic == 0), stop=(ic == DT_CH - 1))
            dst = qt_p.tile([128, G], BF16, tag=f"qT{dt_i}",
                            name=f"qT_{b}_{dt_i}")
            nc.scalar.copy(dst, psq)
            QT.append(dst)
        state[b] = {"QT": QT, "dT": dT, "V": dn16, "mbT": mbT}

    def stage_b(b):
        st = state[b]
        QT, dT, V, mbT = st["QT"], st["dT"], st["V"], st["mbT"]
        pt = pt_p.tile([128, nkt * G], BF16, tag="pt", name=f"pt_{b}")
        HT = [ht_p.tile([128, G], BF16, tag=f"hT{i}", name=f"hT_{b}_{i}")
              for i in range(DT_CH)]
        l_row = misc_p.tile([1, G], F32, tag="l_row", name=f"lrow_{b}")

        def emit_s(kt_i):
            pss = ps_s.tile([128, 1024], F32, tag="ps_s",
                            name=f"pss_{b}_{kt_i}")
            for h in range(2):
                for ic in range(DT_CH):
                    nc.tensor.matmul(
                        pss[:, h * 512:(h + 1) * 512],
                        dT[ic][:, kt_i * 128:(kt_i + 1) * 128],
                        QT[ic][:, h * 512:(h + 1) * 512],
                        start=(ic == 0), stop=(ic == DT_CH - 1))
            nc.scalar.activation(
                out=pt[:, kt_i * G:(kt_i + 1) * G], in_=pss,
                func=mybir.ActivationFunctionType.Exp,
                bias=mbT[:, kt_i:kt_i + 1], scale=NORM)

        def pv_pass(h):
            psH = [ps_acc.tile([128, 512], F32, tag="ps_acc",
                               name=f"psH_{b}_{h}_{i}") for i in range(DT_CH)]
            def emit_pv(kt_i):
                for dt_i in range(DT_CH):
                    nc.tensor.matmul(
                        psH[dt_i],
                        V[kt_i][:, dt_i * 128:(dt_i + 1) * 128],
                        pt[:, kt_i * G + h * 512:kt_i * G + (h + 1) * 512],
                        start=(kt_i == 0), stop=(kt_i == nkt - 1))
            return psH, emit_pv

        psH0, emit_pv0 = pv_pass(0)
        emit_s(0)
        for kt_i in range(1, nkt):
            emit_s(kt_i)
            emit_pv0(kt_i - 1)
        emit_pv0(nkt - 1)

        psl0 = ps_l.tile([1, 512], F32, tag="ps_l", name=f"psl_{b}_0")
        for kt_i in range(nkt):
            nc.tensor.matmul(psl0, ones, pt[:, kt_i * G:kt_i * G + 512],
                             start=(kt_i == 0), stop=(kt_i == nkt - 1))
        for dt_i in range(DT_CH):
            nc.vector.tensor_copy(HT[dt_i][:, 0:512], psH0[dt_i])

        psH1, emit_pv1 = pv_pass(1)
        for kt_i in range(nkt):
            emit_pv1(kt_i)
        nc.scalar.copy(l_row[:, 0:512], psl0)
        psl1 = ps_l.tile([1, 512], F32, tag="ps_l", name=f"psl_{b}_1")
        for kt_i in range(nkt):
            nc.tensor.matmul(psl1, ones,
                             pt[:, kt_i * G + 512:kt_i * G + 1024],
                             start=(kt_i == 0), stop=(kt_i == nkt - 1))
        for dt_i in range(DT_CH):
            nc.scalar.copy(HT[dt_i][:, 512:1024], psH1[dt_i])
        nc.scalar.copy(l_row[:, 512:1024], psl1)

        ps_inv = ps_q.tile([128, 1024], F32, tag="ps_q", name=f"psinv_{b}")
        for j in range(KD):
            nc.tensor.transpose(
                ps_inv[:, j:j + 1], l_row[:, j * 128:(j + 1) * 128],
                ident_f[:1, :1])
        invl = misc_p.tile([128, KD], F32, tag="invl", name=f"invl_{b}")
        nc.vector.reciprocal(invl, ps_inv[:, :KD])
        st["HT"] = HT
        st["invl"] = invl

    def stage_c(b):
        st = state[b]
        HT, invl = st["HT"], st["invl"]
        row0 = b * G

        for p_i in range(KD // 2):
            ps = ps_acc.tile([128, 512], F32, tag="ps_acc",
                             name=f"psf_{b}_{p_i}")
            for j in range(2):
                qt_i = p_i * 2 + j
                for dt_i in range(DT_CH):
                    nc.tensor.matmul(
                        ps[:, j * D:(j + 1) * D],
                        HT[dt_i][:, qt_i * 128:(qt_i + 1) * 128],
                        pto[dt_i],
                        start=(dt_i == 0), stop=(dt_i == DT_CH - 1))
            for j in range(2):
                qt_i = p_i * 2 + j
                ot = out_p.tile([128, D], F32, tag="outp", name=f"ot_{b}_{qt_i}")
                nc.vector.scalar_tensor_tensor(
                    out=ot, in0=ps[:, j * D:(j + 1) * D],
                    scalar=invl[:, qt_i:qt_i + 1], in1=bias_rep,
                    op0=mybir.AluOpType.mult, op1=mybir.AluOpType.add)
                (nc.sync if qt_i % 2 == 0 else nc.gpsimd).dma_start(
                    out=out_d[row0 + qt_i * 128:row0 + (qt_i + 1) * 128, :],
                    in_=ot)
        del state[b]

    if reps > 1:
        loop_cm = tc.For_i(0, reps, 1)
        loop_cm.__enter__()

    stage_a(0)
    for b in range(bpc):
        stage_b(b)
        if b + 1 < bpc:
            stage_a(b + 1)
        stage_c(b)

    if reps > 1:
        loop_cm.__exit__(None, None, None)


# ---------------------------------------------------------------------------
# Host side: compaction + a cached jax.jit(shard_map) runner over the 8 cores.
def compact(data, mask):
    """Per-batch stable-sort of rows so unmasked keys come first."""
    nb = mask.shape[0]
    datac = np.empty_like(data)
    maskc = np.empty_like(mask)
    perms = np.empty((nb, G), np.int64)
    for b in range(nb):
        p = np.argsort(mask[b], kind="stable")
        datac[b * G:(b + 1) * G] = data[b * G:(b + 1) * G][p]
        maskc[b] = mask[b][p]
        perms[b] = p
    return datac, maskc, perms


_RUNNER_CACHE = {}


def _make_runner(nkt):
    import jax
    from jax.experimental.shard_map import shard_map
    from jax.sharding import Mesh, NamedSharding, PartitionSpec

    from concourse.bass2jax import (
        _bass_exec_p,
        install_neuronx_cc_hook,
        partition_id_tensor,
    )

    nc = build_program(nkt)
    install_neuronx_cc_hook()
    assert nc.dbg_addr is None
    partition_name = (nc.partition_id_tensor.name
                      if nc.partition_id_tensor else None)

    in_names, out_names, out_avals, zero_outs = [], [], [], []
    for alloc in nc.m.functions[0].allocations:
        if not isinstance(alloc, mybir.MemoryLocationSet):
            continue
        name = alloc.memorylocations[0].name
        if alloc.kind == "ExternalInput":
            if name != partition_name:
                in_names.append(name)
        elif alloc.kind == "ExternalOutput":
            shape = tuple(alloc.tensor_shape)
            dtype = mybir.dt.np(alloc.dtype)
            out_names.append(name)
            out_avals.append(jax.core.ShapedArray(shape, dtype))
            zero_outs.append(np.zeros((N_CORES * shape[0],) + shape[1:], dtype))
    n_params = len(in_names)
    all_in_names = list(in_names) + list(out_names)
    if partition_name is not None:
        all_in_names.append(partition_name)

    def _body_fn(*args):
        operands = list(args)
        if partition_name is not None:
            operands.append(partition_id_tensor())
        outs = _bass_exec_p.bind(
            *operands,
            out_avals=tuple(out_avals),
            in_names=tuple(all_in_names),
            out_names=tuple(out_names),
            lowering_input_output_aliases=(),
            sim_require_finite=False,
            sim_require_nnan=False,
            nc=nc,
        )
        return tuple(outs)

    devices = jax.devices()[:N_CORES]
    mesh = Mesh(np.asarray(devices), ("core",))
    in_specs = (PartitionSpec("core"),) * (n_params + len(out_names))
    out_specs = (PartitionSpec("core"),) * len(out_names)
    sharded = jax.jit(
        shard_map(_body_fn, mesh=mesh, in_specs=in_specs, out_specs=out_specs,
                  check_rep=False),
        keep_unused=True,
    )
    sharding = NamedSharding(mesh, PartitionSpec("core"))
    dev_zeros = [jax.device_put(z, sharding) for z in zero_outs]
    return {
        "nc": nc, "fn": sharded, "in_names": in_names,
        "out_names": out_names, "sharding": sharding, "dev_zeros": dev_zeros,
    }


def get_runner(nkt=NKT):
    if nkt not in _RUNNER_CACHE:
        _RUNNER_CACHE[nkt] = _make_runner(nkt)
    return _RUNNER_CACHE[nkt]


def _concat_inputs(data, mask, wq, wk, wv, wo, b):
    """Per-core shards concatenated on axis 0, keyed by dram tensor name."""
    return {
        "data": data,
        "mask": mask,
        "w_query": np.concatenate([wq] * N_CORES, axis=0),
        "w_key": np.concatenate([wk] * N_CORES, axis=0),
        "w_val": np.concatenate([wv] * N_CORES, axis=0),
        "w_out": np.concatenate([wo] * N_CORES, axis=0),
        "b_out": np.concatenate([b] * N_CORES, axis=0),
    }


def kernel(data, mask, graph_size, evaluate, W_query, W_key, W_val, W_out,
           b_out, **_ignored):
    data = np.ascontiguousarray(np.asarray(data, dtype=np.float32))
    mask = np.ascontiguousarray(np.asarray(mask, dtype=np.int32))
    wq = np.ascontiguousarray(np.asarray(W_query, dtype=np.float32))
    wk = np.ascontiguousarray(np.asarray(W_key, dtype=np.float32))
    wv = np.ascontiguousarray(np.asarray(W_val, dtype=np.float32))
    wo = np.ascontiguousarray(np.asarray(W_out, dtype=np.float32))
    b = np.ascontiguousarray(np.asarray(b_out, dtype=np.float32))

    datac, maskc, perms = compact(data, mask)
    nk_max = int((G - maskc.sum(axis=1)).max())
    nkt = max(NKT, -(-nk_max // 128))   # ceil; >=NKT so the cached program wins

    r = get_runner(nkt)
    cat = _concat_inputs(datac, maskc, wq, wk, wv, wo, b)
    args = [cat[n] for n in r["in_names"]] + list(r["dev_zeros"])
    outs = r["fn"](*args)
    outc = np.asarray(outs[r["out_names"].index("out")])

    out = np.empty_like(outc)
    rows = (perms + (np.arange(B)[:, None] * G)).reshape(-1)
    out[rows] = outc
    return out
